# revision 31
# baseline (speedup 1.0000x reference)
"""Trainium2 Bass kernel for the ELGCA block (dwconv3x3+gelu || conv1x1+gelu
-> pooled linear attention), data-parallel over batch on 8 NeuronCores.

Self-contained: hardcodes shapes B=16, C=128, H=W=128, f32.
kernel(**inputs) takes full unsharded inputs, returns the FULL f32 output.

v4 (per core, BPC=2 local images, partitions p = b*64 + c):
  - dwconv3x3 on PE: 9 accumulating matmuls per 512-col chunk with
    diagonal bf16 weights, tap-major across each 16-row window (4
    matmuls per LDWEIGHTS — weight swaps inside accumulation chains
    stall the PE ~160ns, so amortize them), f32 PSUM accumulate.
  - conv1x1: both-batch block-diagonal matmuls; A-side (q|k) in f32
    (softmax logits need it), B-side (v|l) bf16.
  - bulk DMA on the GpSimd SWDGE queue (spreads over all 16 DMA
    engines; the two HWDGE queues share one engine pair), small
    outputs on sync/scalar HWDGE.
  - constants loaded with few descriptors (whole-tensor loads + PE
    transposes + on-chip block builds) — scattered tiny-descriptor
    DMAs serialize the queues for tens of us.
  - all outputs written bf16, widened to f32 on the host.
"""

import numpy as np
from contextlib import ExitStack

import concourse.bass as bass
import concourse.tile as tile
from concourse import bacc, mybir
from concourse import bass_utils
from concourse.masks import make_identity

F32 = mybir.dt.float32
BF16 = mybir.dt.bfloat16
AX = mybir.AxisListType
ALU = mybir.AluOpType
ACTF = mybir.ActivationFunctionType

N_CORES = 8
B_TOT, C, H, W = 16, 128, 128, 128
BPC = B_TOT // N_CORES          # 2 images per core
HW = H * W                      # 16384
C2 = C // 2                     # 64
C4 = C // 4                     # 32
WP = W + 2                      # padded row width (130)
NW = 8                          # number of 16-row windows
WR = H // NW                    # image rows per window (16)
NP = (H // 2) * (W // 2)        # 4096 pooled positions
W2 = W // 2                     # 64


def build_nc(loops=1):
    nc = bacc.Bacc("TRN2", target_bir_lowering=False, debug=False,
                   num_devices=N_CORES)
    x = nc.dram_tensor("x", [BPC, C, H, W], F32, kind="ExternalInput").ap()
    dw_w = nc.dram_tensor("dw_w", [C2, 1, 3, 3], F32, kind="ExternalInput").ap()
    dw_b = nc.dram_tensor("dw_b", [C2], F32, kind="ExternalInput").ap()
    qw = nc.dram_tensor("qkvl_w", [C, C2, 1, 1], F32, kind="ExternalInput").ap()
    qb = nc.dram_tensor("qkvl_b", [C], F32, kind="ExternalInput").ap()
    out = nc.dram_tensor("out", [BPC, C, H, W], BF16, kind="ExternalOutput").ap()

    with tile.TileContext(nc) as tc, ExitStack() as ctx:
        consts = ctx.enter_context(tc.tile_pool(name="consts", bufs=1))
        inp = ctx.enter_context(tc.tile_pool(name="inp", bufs=3))
        slabp = ctx.enter_context(tc.tile_pool(name="slabp", bufs=3))
        bigp = ctx.enter_context(tc.tile_pool(name="bigp", bufs=1))
        stgp = ctx.enter_context(tc.tile_pool(name="stgp", bufs=2))
        ps = ctx.enter_context(tc.tile_pool(name="ps", bufs=1, space="PSUM"))

        # ---------------- constants (few-descriptor loads) ----------------
        id_f32 = consts.tile([128, 128], F32)
        make_identity(nc, id_f32[:])

        # dw weights: [64, 9] rows -> dup to [128, 9]
        w_tile = consts.tile([128, 9], F32)
        dw9 = dw_w.rearrange("c o kh kw -> c (o kh kw)")
        nc.gpsimd.dma_start(w_tile[0:C2, :], dw9)
        nc.gpsimd.dma_start(w_tile[C2:128, :], dw9)

        # qkvl_w: load [128oc, 64ic] contiguous, PE-transpose to [64ic, 128oc]
        qw_oc = consts.tile([128, C2], F32)
        nc.gpsimd.dma_start(qw_oc[:], qw.rearrange("o i kh kw -> o (i kh kw)"))
        qwT_ps = ps.tile([128, 1024], F32, tag="cv", bufs=2)
        nc.tensor.transpose(qwT_ps[0:C2, 0:128], qw_oc[:], id_f32[:])
        qwT_sb = consts.tile([C2, 128], F32)
        nc.scalar.copy(qwT_sb[:], qwT_ps[0:C2, 0:128])

        # row-vector loads (1 descriptor each) for biases
        qb_row = consts.tile([1, C], F32)
        nc.gpsimd.dma_start(qb_row[:], qb.unsqueeze(0))
        dwb_row = consts.tile([1, C2], F32)
        nc.gpsimd.dma_start(dwb_row[:], dw_b.unsqueeze(0))

        # permuted bias rows -> PE transpose to per-partition columns
        # brow col-blocks: biasA = [qb0:32|qb0:32|qb32:64|qb32:64],
        # biasB = [qb64:96|...], dwb = [dwb|dwb]
        brow = consts.tile([1, 3 * 128], F32)
        nc.scalar.copy(brow[:, 0:C4], qb_row[:, 0:C4])
        nc.scalar.copy(brow[:, C4:C2], qb_row[:, 0:C4])
        nc.scalar.copy(brow[:, C2:96], qb_row[:, C4:C2])
        nc.scalar.copy(brow[:, 96:128], qb_row[:, C4:C2])
        nc.scalar.copy(brow[:, 128:160], qb_row[:, C2:96])
        nc.scalar.copy(brow[:, 160:192], qb_row[:, C2:96])
        nc.scalar.copy(brow[:, 192:224], qb_row[:, 96:128])
        nc.scalar.copy(brow[:, 224:256], qb_row[:, 96:128])
        nc.scalar.copy(brow[:, 256:320], dwb_row[:, 0:C2])
        nc.scalar.copy(brow[:, 320:384], dwb_row[:, 0:C2])
        bcol_ps = ps.tile([128, 1024], F32, tag="dwA")
        for i in range(3):
            nc.tensor.transpose(bcol_ps[:, i:i + 1],
                                brow[0:1, i * 128:(i + 1) * 128],
                                id_f32[0:1, 0:1])
        bcol = consts.tile([128, 3], F32)
        nc.scalar.copy(bcol[:], bcol_ps[:, 0:3])
        biasA = bcol[:, 0:1]
        biasB = bcol[:, 1:2]
        dwb_t = bcol[:, 2:3]

        # 9 diagonal tap matrices, bf16
        wdiag_f = consts.tile([128, 9 * 128], F32)
        wdiag = consts.tile([128, 9 * 128], BF16)
        for t in range(9):
            nc.vector.tensor_scalar_mul(
                wdiag_f[:, t * 128:(t + 1) * 128], id_f32[:],
                w_tile[:, t:t + 1])
        nc.vector.tensor_copy(wdiag[:], wdiag_f[:])

        # conv1x1 block-diagonal weights from qwT_sb (on-chip copies)
        lhs_f = consts.tile([128, 256], F32)
        nc.vector.memset(lhs_f[:], 0.0)
        nc.scalar.copy(lhs_f[0:C2, 0:C4], qwT_sb[:, 0:C4])
        nc.scalar.copy(lhs_f[C2:128, C4:C2], qwT_sb[:, 0:C4])
        nc.scalar.copy(lhs_f[0:C2, C2:96], qwT_sb[:, C4:C2])
        nc.scalar.copy(lhs_f[C2:128, 96:128], qwT_sb[:, C4:C2])
        nc.scalar.copy(lhs_f[0:C2, 128:160], qwT_sb[:, C2:96])
        nc.scalar.copy(lhs_f[C2:128, 160:192], qwT_sb[:, C2:96])
        nc.scalar.copy(lhs_f[0:C2, 192:224], qwT_sb[:, 96:128])
        nc.scalar.copy(lhs_f[C2:128, 224:256], qwT_sb[:, 96:128])
        lhsAB = consts.tile([128, 256], BF16)
        nc.vector.tensor_copy(lhsAB[:], lhs_f[:])
        id_bf = consts.tile([128, 128], BF16)
        nc.vector.tensor_copy(id_bf[:], id_f32[:])
        lhsA = lhs_f[:, 0:128]          # f32: qk logits need full precision
        lhsB = lhsAB[:, 128:256]

        def one_pass():
            # persistent per-pass buffers
            vl = bigp.tile([128, HW], BF16, tag="vl")   # v rows 0:64, l 64:128
            hp = bigp.tile([128, H * W2], F32, tag="hp")
            hp3 = hp.rearrange("p (r w) -> p r w", w=W2)
            hpv = hp.rearrange("p (o two w) -> p o two w", two=2, w=W2)
            pp = bigp.tile([128, NP], F32, tag="pp")
            pp3 = pp.rearrange("p (r w) -> p r w", w=W2)
            qk_acc = bigp.tile([C2, C2], F32, tag="qk")

            def issue_inputs(w):
                """DMA window w's inputs and return (x2tmp, x2bf, slab3)."""
                y0 = w * WR
                ys = max(y0 - 1, 0)
                ye = min(y0 + WR + 1, H)
                nrows = ye - ys
                rs = 0 if w > 0 else 1
                xtmp = inp.tile([128, 18 * W], F32, tag="xtmp")
                xtmp3 = xtmp.rearrange("p (r w) -> p r w", w=W)
                if w <= 1:
                    rh = nrows // 2
                    for b in range(2):
                        nc.gpsimd.dma_start(
                            xtmp3[C2 * b:C2 * b + C2, 0:rh, :],
                            x[b:b + 1, 0:C2, ys:ys + rh, :])
                        nc.gpsimd.dma_start(
                            xtmp3[C2 * b:C2 * b + C2, rh:nrows, :],
                            x[b:b + 1, 0:C2, ys + rh:ye, :])
                else:
                    nc.gpsimd.dma_start(xtmp3[0:C2, 0:nrows, :],
                                        x[0:1, 0:C2, ys:ye, :])
                    nc.gpsimd.dma_start(xtmp3[C2:128, 0:nrows, :],
                                        x[1:2, 0:C2, ys:ye, :])
                x2tmp = inp.tile([128, WR * W], F32, tag="x2tmp")
                x2tmp3 = x2tmp.rearrange("p (r w) -> p r w", w=W)
                nc.gpsimd.dma_start(x2tmp3[0:C2, :, :],
                                    x[0:1, C2:C, y0:y0 + WR, :])
                nc.gpsimd.dma_start(x2tmp3[C2:128, :, :],
                                    x[1:2, C2:C, y0:y0 + WR, :])
                slab = slabp.tile([128, 18 * WP], BF16, tag="slab")
                slab3 = slab.rearrange("p (r w) -> p r w", w=WP)
                nc.gpsimd.memset(slab3[:, :, 0:1], 0.0)
                nc.gpsimd.memset(slab3[:, :, WP - 1:WP], 0.0)
                if w == 0:
                    nc.gpsimd.memset(slab3[:, 0:1, :], 0.0)
                if w == NW - 1:
                    nc.gpsimd.memset(slab3[:, 17:18, :], 0.0)
                nc.vector.tensor_copy(slab3[:, rs:rs + nrows, 1:W + 1],
                                      xtmp3[:, 0:nrows, :])
                x2bf = inp.tile([128, WR * W], BF16, tag="x2bf")
                nc.vector.tensor_copy(x2bf[:], x2tmp[:])
                return x2tmp, x2bf, slab3

            def attn_slice(sw):
                """v-pool pooled rows [8sw, 8sw+8), transpose the 4 new
                128-position chunks, accumulate their qk partial."""
                o0 = 8 * sw
                nc.vector.tensor_add(pp3[0:C2, o0:o0 + 8, :],
                                     hpv[0:C2, o0:o0 + 8, 0, :],
                                     hpv[0:C2, o0:o0 + 8, 1, :])
                lo = max(o0, 1)
                nc.vector.tensor_add(pp3[0:C2, lo:o0 + 8, :],
                                     pp3[0:C2, lo:o0 + 8, :],
                                     hpv[0:C2, lo - 1:o0 + 7, 1, :])
                nc.vector.tensor_max(pp3[C2:128, o0:o0 + 8, :],
                                     hpv[C2:128, o0:o0 + 8, 0, :],
                                     hpv[C2:128, o0:o0 + 8, 1, :])
                trps = ps.tile([128, 1024], F32, tag="cv", bufs=2)
                for jj in range(4):
                    ch = 4 * sw + jj
                    nc.tensor.transpose(trps[:, jj * 128:(jj + 1) * 128],
                                        pp[:, ch * 128:(ch + 1) * 128],
                                        id_f32[:])
                trsb = stgp.tile([128, 512], F32, tag="trsb")
                nc.scalar.copy(trsb[:], trps[:, 0:512])
                qkps = ps.tile([128, 1024], F32, tag="cv", bufs=2)
                for jj in range(4):
                    nc.tensor.matmul(
                        qkps[0:C2, 0:C2],
                        trsb[:, jj * 128 + C2:(jj + 1) * 128],
                        trsb[:, jj * 128:jj * 128 + C2],
                        start=(jj == 0), stop=(jj == 3))
                if sw == 0:
                    nc.scalar.copy(qk_acc[:], qkps[0:C2, 0:C2])
                else:
                    nc.vector.tensor_add(qk_acc[:], qk_acc[:],
                                         qkps[0:C2, 0:C2])

            pend = [issue_inputs(0), issue_inputs(1)]
            for w in range(NW):
                y0 = w * WR
                x2tmp, x2bf, slab3 = pend.pop(0)
                if w + 2 < NW:
                    pend.append(issue_inputs(w + 2))

                qg = stgp.tile([128, WR * W], F32, tag="qg", bufs=1)
                x1st = stgp.tile([128, WR * W], BF16, tag="x1st")

                # ---- incremental attn for the previous window ----
                if w >= 1:
                    attn_slice(w - 1)

                # ---- dwconv: taps 0-5 on PE (tap-major), taps 6-8
                # (dy=2) fused on DVE, merged via an identity matmul ----
                dwacc = stgp.tile([128, WR * W], BF16, tag="dwacc", bufs=1)
                acc3 = dwacc.rearrange("p (r w) -> p r w", w=W)
                nc.vector.tensor_scalar_mul(acc3[:],
                                            slab3[:, 2:2 + WR, 0:W],
                                            w_tile[:, 6:7])
                for t in (7, 8):
                    dx = t % 3
                    nc.vector.scalar_tensor_tensor(
                        acc3[:], slab3[:, 2:2 + WR, dx:dx + W],
                        w_tile[:, t:t + 1], acc3[:],
                        op0=ALU.mult, op1=ALU.add)
                dwA = ps.tile([128, 1024], F32, tag="dwA")
                dwB = ps.tile([128, 1024], F32, tag="dwB")
                for t in range(6):
                    dy, dx = t // 3, t % 3
                    for q in range(4):
                        tgt = dwA if q < 2 else dwB
                        la = q * 4
                        nc.tensor.matmul(
                            tgt[:, (q % 2) * 512:(q % 2 + 1) * 512],
                            wdiag[:, t * 128:(t + 1) * 128],
                            slab3[:, la + dy:la + dy + 4, dx:dx + W],
                            start=(t == 0), stop=False)
                for q in range(4):
                    tgt = dwA if q < 2 else dwB
                    nc.tensor.matmul(
                        tgt[:, (q % 2) * 512:(q % 2 + 1) * 512], id_bf[:],
                        dwacc[:, q * 512:(q + 1) * 512],
                        start=False, stop=True)
                nc.scalar.activation(x1st[:, 0:1024], dwA[:], ACTF.Gelu,
                                     bias=dwb_t)
                nc.scalar.activation(x1st[:, 1024:2048], dwB[:], ACTF.Gelu,
                                     bias=dwb_t)
                x1st3 = x1st.rearrange("p (r w) -> p r w", w=W)
                nc.gpsimd.dma_start(out[0:1, 0:C2, y0:y0 + WR, :],
                                    x1st3[0:C2, :, :])
                nc.gpsimd.dma_start(out[1:2, 0:C2, y0:y0 + WR, :],
                                    x1st3[C2:128, :, :])

                # ---- conv1x1: A pairs (f32) then B pairs (bf16) ----
                for pr in range(2):
                    pc0 = pr * 1024
                    Aps = ps.tile([128, 1024], F32, tag="cv", bufs=2)
                    for hf in range(2):
                        nc.tensor.matmul(
                            Aps[:, hf * 512:(hf + 1) * 512], lhsA,
                            x2tmp[:, pc0 + hf * 512:pc0 + (hf + 1) * 512],
                            start=True, stop=True)
                    nc.scalar.activation(qg[:, pc0:pc0 + 1024], Aps[:],
                                         ACTF.Gelu, bias=biasA)
                for pr in range(2):
                    pc0 = pr * 1024
                    Bps = ps.tile([128, 1024], F32, tag="cv", bufs=2)
                    for hf in range(2):
                        nc.tensor.matmul(
                            Bps[:, hf * 512:(hf + 1) * 512], lhsB,
                            x2bf[:, pc0 + hf * 512:pc0 + (hf + 1) * 512],
                            start=True, stop=True)
                    nc.scalar.activation(vl[:, y0 * W + pc0:
                                            y0 * W + pc0 + 1024],
                                         Bps[:], ACTF.Gelu, bias=biasB)

                # ---- l output DMA for this window ----
                nc.sync.dma_start(
                    out[0:BPC, C2:96, y0:y0 + WR, :],
                    vl[C2:128, y0 * W:(y0 + WR) * W]
                    .rearrange("p (r w) -> p r w", w=W))

                # ---- horizontal pooling for this window ----
                qg3 = qg.rearrange("p (r w2 two) -> p r w2 two", two=2, w2=W2)
                nc.gpsimd.tensor_add(hp3[0:C2, y0:y0 + WR, :],
                                     qg3[0:C2, :, :, 0], qg3[0:C2, :, :, 1])
                nc.gpsimd.tensor_add(hp3[0:C2, y0:y0 + WR, 1:W2],
                                     hp3[0:C2, y0:y0 + WR, 1:W2],
                                     qg3[0:C2, :, 0:W2 - 1, 1])
                nc.vector.tensor_max(hp3[C2:128, y0:y0 + WR, :],
                                     qg3[C2:128, :, :, 0],
                                     qg3[C2:128, :, :, 1])


            attn_slice(NW - 1)

            # ---------- softmax stats -> block-diag attention ----------
            Ebd = bigp.tile([C2, C2], BF16, tag="Ebd")
            nc.vector.memset(Ebd[:], 0.0)
            qk9 = bigp.tile([C2, C2], F32, tag="qk9")
            nc.scalar.mul(qk9[:], qk_acc[:], 1.0 / 9.0)
            for bi in range(BPC):
                o = C4 * bi
                blk = qk9[o:o + C4, o:o + C4]
                nmax = bigp.tile([C4, 1], F32, tag=f"nmax{bi}")
                nc.vector.tensor_reduce(nmax[:], blk, axis=AX.X,
                                        op=ALU.max, negate=True)
                ET = bigp.tile([C4, C4], F32, tag=f"ET{bi}")
                nc.scalar.activation(ET[:], blk, ACTF.Exp,
                                     bias=nmax[:, 0:1])
                ssum = bigp.tile([C4, 1], F32, tag=f"ssum{bi}")
                nc.vector.reduce_sum(ssum[:], ET[:], axis=AX.X)
                rec = bigp.tile([C4, 1], F32, tag=f"rec{bi}")
                nc.vector.reciprocal(rec[:], ssum[:])
                ETn = bigp.tile([C4, C4], F32, tag=f"ETn{bi}")
                nc.vector.tensor_scalar_mul(ETn[:], ET[:], rec[:, 0:1])
                etp = ps.tile([128, 1024], F32, tag="dwB")
                nc.tensor.transpose(etp[0:C4, 0:C4], ETn[:],
                                    id_f32[0:C4, 0:C4])
                nc.scalar.copy(Ebd[o:o + C4, o:o + C4], etp[0:C4, 0:C4])

            # ---------- out2 = attn @ v, both batches per matmul ----------
            for w in range(NW):
                y0 = w * WR
                o2st = stgp.tile([C2, WR * W], BF16, tag="o2st")
                for pr in range(2):
                    pc0 = pr * 1024
                    o2ps = ps.tile([128, 1024], F32, tag="cv", bufs=2)
                    for hf in range(2):
                        nc.tensor.matmul(
                            o2ps[0:C2, hf * 512:(hf + 1) * 512], Ebd[:],
                            vl[0:C2, y0 * W + pc0 + hf * 512:
                               y0 * W + pc0 + (hf + 1) * 512],
                            start=True, stop=True)
                    nc.scalar.copy(o2st[:, pc0:pc0 + 1024],
                                   o2ps[0:C2, :])
                o2st3 = o2st.rearrange("p (r w) -> p r w", w=W)
                hr = WR // 2
                for b in range(2):
                    nc.gpsimd.dma_start(
                        out[b:b + 1, 96:128, y0:y0 + hr, :],
                        o2st3[C4 * b:C4 * b + C4, 0:hr, :])
                    nc.gpsimd.dma_start(
                        out[b:b + 1, 96:128, y0 + hr:y0 + WR, :],
                        o2st3[C4 * b:C4 * b + C4, hr:WR, :])

        for _ in range(loops):
            one_pass()

    nc.compile()
    return nc


_NC_CACHE = None


def _get_nc():
    global _NC_CACHE
    if _NC_CACHE is None:
        _NC_CACHE = build_nc()
    return _NC_CACHE


def kernel(x, dw_w, dw_b, qkvl_w, qkvl_b):
    x = np.ascontiguousarray(np.asarray(x, dtype=np.float32))
    shared = {
        "dw_w": np.ascontiguousarray(np.asarray(dw_w, dtype=np.float32)),
        "dw_b": np.ascontiguousarray(np.asarray(dw_b, dtype=np.float32)),
        "qkvl_w": np.ascontiguousarray(np.asarray(qkvl_w, dtype=np.float32)),
        "qkvl_b": np.ascontiguousarray(np.asarray(qkvl_b, dtype=np.float32)),
    }
    nc = _get_nc()
    in_maps = [
        {"x": x[c * BPC:(c + 1) * BPC], **shared} for c in range(N_CORES)
    ]
    res = bass_utils.run_bass_kernel_spmd(nc, in_maps,
                                          core_ids=list(range(N_CORES)))
    return np.concatenate(
        [np.asarray(res.results[c]["out"]).astype(np.float32)
         for c in range(N_CORES)], axis=0)


# revision 33
# speedup vs baseline: 1.0849x; 1.0849x over previous
"""Trainium2 Bass kernel for the ELGCA block (dwconv3x3+gelu || conv1x1+gelu
-> pooled linear attention), data-parallel over batch on 8 NeuronCores.

Self-contained: hardcodes shapes B=16, C=128, H=W=128, f32.
kernel(**inputs) takes full unsharded inputs, returns the FULL f32 output.

v4 (per core, BPC=2 local images, partitions p = b*64 + c):
  - dwconv3x3 on PE: 9 accumulating matmuls per 512-col chunk with
    diagonal bf16 weights, tap-major across each 16-row window (4
    matmuls per LDWEIGHTS — weight swaps inside accumulation chains
    stall the PE ~160ns, so amortize them), f32 PSUM accumulate.
  - conv1x1: both-batch block-diagonal matmuls; A-side (q|k) in f32
    (softmax logits need it), B-side (v|l) bf16.
  - bulk DMA on the GpSimd SWDGE queue (spreads over all 16 DMA
    engines; the two HWDGE queues share one engine pair), small
    outputs on sync/scalar HWDGE.
  - constants loaded with few descriptors (whole-tensor loads + PE
    transposes + on-chip block builds) — scattered tiny-descriptor
    DMAs serialize the queues for tens of us.
  - all outputs written bf16, widened to f32 on the host.
"""

import numpy as np
from contextlib import ExitStack

import concourse.bass as bass
import concourse.tile as tile
from concourse import bacc, mybir
from concourse import bass_utils
from concourse.masks import make_identity

F32 = mybir.dt.float32
BF16 = mybir.dt.bfloat16
AX = mybir.AxisListType
ALU = mybir.AluOpType
ACTF = mybir.ActivationFunctionType

N_CORES = 8
B_TOT, C, H, W = 16, 128, 128, 128
BPC = B_TOT // N_CORES          # 2 images per core
HW = H * W                      # 16384
C2 = C // 2                     # 64
C4 = C // 4                     # 32
WP = W + 2                      # padded row width (130)
NW = 8                          # number of 16-row windows
WR = H // NW                    # image rows per window (16)
NP = (H // 2) * (W // 2)        # 4096 pooled positions
W2 = W // 2                     # 64


def build_nc(loops=1):
    nc = bacc.Bacc("TRN2", target_bir_lowering=False, debug=False,
                   num_devices=N_CORES)
    x = nc.dram_tensor("x", [BPC, C, H, W], F32, kind="ExternalInput").ap()
    dw_w = nc.dram_tensor("dw_w", [C2, 1, 3, 3], F32, kind="ExternalInput").ap()
    dw_b = nc.dram_tensor("dw_b", [C2], F32, kind="ExternalInput").ap()
    qw = nc.dram_tensor("qkvl_w", [C, C2, 1, 1], F32, kind="ExternalInput").ap()
    qb = nc.dram_tensor("qkvl_b", [C], F32, kind="ExternalInput").ap()
    out = nc.dram_tensor("out", [BPC, C, H, W], BF16, kind="ExternalOutput").ap()

    with tile.TileContext(nc) as tc, ExitStack() as ctx:
        consts = ctx.enter_context(tc.tile_pool(name="consts", bufs=1))
        inp = ctx.enter_context(tc.tile_pool(name="inp", bufs=3))
        slabp = ctx.enter_context(tc.tile_pool(name="slabp", bufs=3))
        bigp = ctx.enter_context(tc.tile_pool(name="bigp", bufs=1))
        stgp = ctx.enter_context(tc.tile_pool(name="stgp", bufs=2))
        ps = ctx.enter_context(tc.tile_pool(name="ps", bufs=1, space="PSUM"))

        # ---------------- constants (few-descriptor loads) ----------------
        id_f32 = consts.tile([128, 128], F32)
        make_identity(nc, id_f32[:])

        # dw weights: [64, 9] rows -> dup to [128, 9]
        w_tile = consts.tile([128, 9], F32)
        dw9 = dw_w.rearrange("c o kh kw -> c (o kh kw)")
        nc.gpsimd.dma_start(w_tile[0:C2, :], dw9)
        nc.gpsimd.dma_start(w_tile[C2:128, :], dw9)

        # qkvl_w: load [128oc, 64ic] contiguous, PE-transpose to [64ic, 128oc]
        qw_oc = consts.tile([128, C2], F32)
        nc.gpsimd.dma_start(qw_oc[:], qw.rearrange("o i kh kw -> o (i kh kw)"))
        qwT_ps = ps.tile([128, 1024], F32, tag="cv", bufs=2)
        nc.tensor.transpose(qwT_ps[0:C2, 0:128], qw_oc[:], id_f32[:])
        qwT_sb = consts.tile([C2, 128], F32)
        nc.scalar.copy(qwT_sb[:], qwT_ps[0:C2, 0:128])

        # row-vector loads (1 descriptor each) for biases
        qb_row = consts.tile([1, C], F32)
        nc.gpsimd.dma_start(qb_row[:], qb.unsqueeze(0))
        dwb_row = consts.tile([1, C2], F32)
        nc.gpsimd.dma_start(dwb_row[:], dw_b.unsqueeze(0))

        # permuted bias rows -> PE transpose to per-partition columns
        # brow col-blocks: biasA = [qb0:32|qb0:32|qb32:64|qb32:64],
        # biasB = [qb64:96|...], dwb = [dwb|dwb]
        brow = consts.tile([1, 3 * 128], F32)
        nc.scalar.copy(brow[:, 0:C4], qb_row[:, 0:C4])
        nc.scalar.copy(brow[:, C4:C2], qb_row[:, 0:C4])
        nc.scalar.copy(brow[:, C2:96], qb_row[:, C4:C2])
        nc.scalar.copy(brow[:, 96:128], qb_row[:, C4:C2])
        nc.scalar.copy(brow[:, 128:160], qb_row[:, C2:96])
        nc.scalar.copy(brow[:, 160:192], qb_row[:, C2:96])
        nc.scalar.copy(brow[:, 192:224], qb_row[:, 96:128])
        nc.scalar.copy(brow[:, 224:256], qb_row[:, 96:128])
        nc.scalar.copy(brow[:, 256:320], dwb_row[:, 0:C2])
        nc.scalar.copy(brow[:, 320:384], dwb_row[:, 0:C2])
        bcol_ps = ps.tile([128, 1024], F32, tag="dwA")
        for i in range(3):
            nc.tensor.transpose(bcol_ps[:, i:i + 1],
                                brow[0:1, i * 128:(i + 1) * 128],
                                id_f32[0:1, 0:1])
        bcol = consts.tile([128, 3], F32)
        nc.scalar.copy(bcol[:], bcol_ps[:, 0:3])
        biasA = bcol[:, 0:1]
        biasB = bcol[:, 1:2]
        dwb_t = bcol[:, 2:3]

        # 9 diagonal tap matrices, bf16
        wdiag_f = consts.tile([128, 9 * 128], F32)
        wdiag = consts.tile([128, 9 * 128], BF16)
        for t in range(9):
            nc.vector.tensor_scalar_mul(
                wdiag_f[:, t * 128:(t + 1) * 128], id_f32[:],
                w_tile[:, t:t + 1])
        nc.vector.tensor_copy(wdiag[:], wdiag_f[:])

        # conv1x1 block-diagonal weights from qwT_sb (on-chip copies)
        lhs_f = consts.tile([128, 256], F32)
        nc.vector.memset(lhs_f[:], 0.0)
        nc.scalar.copy(lhs_f[0:C2, 0:C4], qwT_sb[:, 0:C4])
        nc.scalar.copy(lhs_f[C2:128, C4:C2], qwT_sb[:, 0:C4])
        nc.scalar.copy(lhs_f[0:C2, C2:96], qwT_sb[:, C4:C2])
        nc.scalar.copy(lhs_f[C2:128, 96:128], qwT_sb[:, C4:C2])
        nc.scalar.copy(lhs_f[0:C2, 128:160], qwT_sb[:, C2:96])
        nc.scalar.copy(lhs_f[C2:128, 160:192], qwT_sb[:, C2:96])
        nc.scalar.copy(lhs_f[0:C2, 192:224], qwT_sb[:, 96:128])
        nc.scalar.copy(lhs_f[C2:128, 224:256], qwT_sb[:, 96:128])
        lhsAB = consts.tile([128, 256], BF16)
        nc.vector.tensor_copy(lhsAB[:], lhs_f[:])
        id_bf = consts.tile([128, 128], BF16)
        nc.vector.tensor_copy(id_bf[:], id_f32[:])
        lhsA = lhs_f[:, 0:128]          # f32: qk logits need full precision
        lhsB = lhsAB[:, 128:256]

        def one_pass():
            # persistent per-pass buffers
            vl = bigp.tile([128, HW], BF16, tag="vl")   # v rows 0:64, l 64:128
            hp = bigp.tile([128, H * W2], F32, tag="hp")
            hp3 = hp.rearrange("p (r w) -> p r w", w=W2)
            hpv = hp.rearrange("p (o two w) -> p o two w", two=2, w=W2)
            pp = bigp.tile([128, NP], F32, tag="pp")
            pp3 = pp.rearrange("p (r w) -> p r w", w=W2)
            qk_acc = bigp.tile([C2, C2], F32, tag="qk")

            def issue_inputs(w):
                """DMA window w's inputs and return (x2tmp, x2bf, slab3)."""
                y0 = w * WR
                ys = max(y0 - 1, 0)
                ye = min(y0 + WR + 1, H)
                nrows = ye - ys
                rs = 0 if w > 0 else 1
                xtmp = inp.tile([128, 18 * W], F32, tag="xtmp")
                xtmp3 = xtmp.rearrange("p (r w) -> p r w", w=W)
                if w <= 1:
                    rh = nrows // 2
                    for b in range(2):
                        nc.gpsimd.dma_start(
                            xtmp3[C2 * b:C2 * b + C2, 0:rh, :],
                            x[b:b + 1, 0:C2, ys:ys + rh, :])
                        nc.gpsimd.dma_start(
                            xtmp3[C2 * b:C2 * b + C2, rh:nrows, :],
                            x[b:b + 1, 0:C2, ys + rh:ye, :])
                else:
                    nc.gpsimd.dma_start(xtmp3[0:C2, 0:nrows, :],
                                        x[0:1, 0:C2, ys:ye, :])
                    nc.gpsimd.dma_start(xtmp3[C2:128, 0:nrows, :],
                                        x[1:2, 0:C2, ys:ye, :])
                x2tmp = inp.tile([128, WR * W], F32, tag="x2tmp")
                x2tmp3 = x2tmp.rearrange("p (r w) -> p r w", w=W)
                nc.gpsimd.dma_start(x2tmp3[0:C2, :, :],
                                    x[0:1, C2:C, y0:y0 + WR, :])
                nc.gpsimd.dma_start(x2tmp3[C2:128, :, :],
                                    x[1:2, C2:C, y0:y0 + WR, :])
                slab = slabp.tile([128, 18 * WP], BF16, tag="slab")
                slab3 = slab.rearrange("p (r w) -> p r w", w=WP)
                nc.gpsimd.memset(slab3[:, :, 0:1], 0.0)
                nc.gpsimd.memset(slab3[:, :, WP - 1:WP], 0.0)
                if w == 0:
                    nc.gpsimd.memset(slab3[:, 0:1, :], 0.0)
                if w == NW - 1:
                    nc.gpsimd.memset(slab3[:, 17:18, :], 0.0)
                nc.vector.tensor_copy(slab3[:, rs:rs + nrows, 1:W + 1],
                                      xtmp3[:, 0:nrows, :])
                x2bf = inp.tile([128, WR * W], BF16, tag="x2bf")
                nc.scalar.copy(x2bf[:], x2tmp[:])
                return x2tmp, x2bf, slab3

            def attn_slice(sw):
                """v-pool pooled rows [8sw, 8sw+8), transpose the 4 new
                128-position chunks, accumulate their qk partial."""
                o0 = 8 * sw
                nc.vector.tensor_add(pp3[0:C2, o0:o0 + 8, :],
                                     hpv[0:C2, o0:o0 + 8, 0, :],
                                     hpv[0:C2, o0:o0 + 8, 1, :])
                lo = max(o0, 1)
                nc.vector.tensor_add(pp3[0:C2, lo:o0 + 8, :],
                                     pp3[0:C2, lo:o0 + 8, :],
                                     hpv[0:C2, lo - 1:o0 + 7, 1, :])
                nc.vector.tensor_max(pp3[C2:128, o0:o0 + 8, :],
                                     hpv[C2:128, o0:o0 + 8, 0, :],
                                     hpv[C2:128, o0:o0 + 8, 1, :])
                trps = ps.tile([128, 1024], F32, tag="cv", bufs=2)
                for jj in range(4):
                    ch = 4 * sw + jj
                    nc.tensor.transpose(trps[:, jj * 128:(jj + 1) * 128],
                                        pp[:, ch * 128:(ch + 1) * 128],
                                        id_f32[:])
                trsb = stgp.tile([128, 512], F32, tag="trsb")
                nc.scalar.copy(trsb[:], trps[:, 0:512])
                qkps = ps.tile([128, 1024], F32, tag="cv", bufs=2)
                for jj in range(4):
                    nc.tensor.matmul(
                        qkps[0:C2, 0:C2],
                        trsb[:, jj * 128 + C2:(jj + 1) * 128],
                        trsb[:, jj * 128:jj * 128 + C2],
                        start=(jj == 0), stop=(jj == 3))
                if sw == 0:
                    nc.scalar.copy(qk_acc[:], qkps[0:C2, 0:C2])
                else:
                    nc.vector.tensor_add(qk_acc[:], qk_acc[:],
                                         qkps[0:C2, 0:C2])

            pend = [issue_inputs(0), issue_inputs(1)]
            for w in range(NW):
                y0 = w * WR
                x2tmp, x2bf, slab3 = pend.pop(0)
                if w + 2 < NW:
                    pend.append(issue_inputs(w + 2))

                qg = stgp.tile([128, WR * W], F32, tag="qg", bufs=1)
                x1st = stgp.tile([128, WR * W], BF16, tag="x1st")

                # ---- incremental attn for the previous window ----
                if w >= 1:
                    attn_slice(w - 1)

                # ---- dwconv: taps 0-5 on PE (tap-major), taps 6-8
                # (dy=2) fused on DVE, merged via an identity matmul ----
                dwacc = stgp.tile([128, WR * W], BF16, tag="dwacc", bufs=1)
                acc3 = dwacc.rearrange("p (r w) -> p r w", w=W)
                for hh in range(2):
                    r0 = hh * 8
                    nc.vector.tensor_scalar_mul(
                        acc3[:, r0:r0 + 8, :],
                        slab3[:, 2 + r0:2 + r0 + 8, 0:W],
                        w_tile[:, 6:7])
                    for t in (7, 8):
                        dx = t % 3
                        nc.vector.scalar_tensor_tensor(
                            acc3[:, r0:r0 + 8, :],
                            slab3[:, 2 + r0:2 + r0 + 8, dx:dx + W],
                            w_tile[:, t:t + 1], acc3[:, r0:r0 + 8, :],
                            op0=ALU.mult, op1=ALU.add)
                dwA = ps.tile([128, 1024], F32, tag="dwA")
                dwB = ps.tile([128, 1024], F32, tag="dwB")
                for t in range(6):
                    dy, dx = t // 3, t % 3
                    for q in range(4):
                        tgt = dwA if q < 2 else dwB
                        la = q * 4
                        nc.tensor.matmul(
                            tgt[:, (q % 2) * 512:(q % 2 + 1) * 512],
                            wdiag[:, t * 128:(t + 1) * 128],
                            slab3[:, la + dy:la + dy + 4, dx:dx + W],
                            start=(t == 0), stop=False)
                for q in range(4):
                    tgt = dwA if q < 2 else dwB
                    nc.tensor.matmul(
                        tgt[:, (q % 2) * 512:(q % 2 + 1) * 512], id_bf[:],
                        dwacc[:, q * 512:(q + 1) * 512],
                        start=False, stop=True)
                nc.scalar.activation(x1st[:, 0:1024], dwA[:], ACTF.Gelu,
                                     bias=dwb_t)
                nc.scalar.activation(x1st[:, 1024:2048], dwB[:], ACTF.Gelu,
                                     bias=dwb_t)
                x1st3 = x1st.rearrange("p (r w) -> p r w", w=W)
                nc.gpsimd.dma_start(out[0:1, 0:C2, y0:y0 + WR, :],
                                    x1st3[0:C2, :, :])
                nc.gpsimd.dma_start(out[1:2, 0:C2, y0:y0 + WR, :],
                                    x1st3[C2:128, :, :])

                # ---- conv1x1: A pairs (f32) then B pairs (bf16) ----
                for pr in range(2):
                    pc0 = pr * 1024
                    Aps = ps.tile([128, 1024], F32, tag="cv", bufs=2)
                    for hf in range(2):
                        nc.tensor.matmul(
                            Aps[:, hf * 512:(hf + 1) * 512], lhsA,
                            x2tmp[:, pc0 + hf * 512:pc0 + (hf + 1) * 512],
                            start=True, stop=True)
                    nc.scalar.activation(qg[:, pc0:pc0 + 1024], Aps[:],
                                         ACTF.Gelu, bias=biasA)
                for pr in range(2):
                    pc0 = pr * 1024
                    Bps = ps.tile([128, 1024], F32, tag="cv", bufs=2)
                    for hf in range(2):
                        nc.tensor.matmul(
                            Bps[:, hf * 512:(hf + 1) * 512], lhsB,
                            x2bf[:, pc0 + hf * 512:pc0 + (hf + 1) * 512],
                            start=True, stop=True)
                    nc.scalar.activation(vl[:, y0 * W + pc0:
                                            y0 * W + pc0 + 1024],
                                         Bps[:], ACTF.Gelu, bias=biasB)

                # ---- l output DMA for this window ----
                nc.sync.dma_start(
                    out[0:BPC, C2:96, y0:y0 + WR, :],
                    vl[C2:128, y0 * W:(y0 + WR) * W]
                    .rearrange("p (r w) -> p r w", w=W))

                # ---- horizontal pooling for this window ----
                qg3 = qg.rearrange("p (r w2 two) -> p r w2 two", two=2, w2=W2)
                nc.gpsimd.tensor_add(hp3[0:C2, y0:y0 + WR, :],
                                     qg3[0:C2, :, :, 0], qg3[0:C2, :, :, 1])
                nc.vector.tensor_add(hp3[0:C2, y0:y0 + WR, 1:W2],
                                     hp3[0:C2, y0:y0 + WR, 1:W2],
                                     qg3[0:C2, :, 0:W2 - 1, 1])
                nc.vector.tensor_max(hp3[C2:128, y0:y0 + WR, :],
                                     qg3[C2:128, :, :, 0],
                                     qg3[C2:128, :, :, 1])


            attn_slice(NW - 1)

            # ---------- softmax stats -> block-diag attention ----------
            Ebd = bigp.tile([C2, C2], BF16, tag="Ebd")
            nc.vector.memset(Ebd[:], 0.0)
            qk9 = bigp.tile([C2, C2], F32, tag="qk9")
            nc.scalar.mul(qk9[:], qk_acc[:], 1.0 / 9.0)
            for bi in range(BPC):
                o = C4 * bi
                blk = qk9[o:o + C4, o:o + C4]
                nmax = bigp.tile([C4, 1], F32, tag=f"nmax{bi}")
                nc.vector.tensor_reduce(nmax[:], blk, axis=AX.X,
                                        op=ALU.max, negate=True)
                ET = bigp.tile([C4, C4], F32, tag=f"ET{bi}")
                nc.scalar.activation(ET[:], blk, ACTF.Exp,
                                     bias=nmax[:, 0:1])
                ssum = bigp.tile([C4, 1], F32, tag=f"ssum{bi}")
                nc.vector.reduce_sum(ssum[:], ET[:], axis=AX.X)
                rec = bigp.tile([C4, 1], F32, tag=f"rec{bi}")
                nc.vector.reciprocal(rec[:], ssum[:])
                ETn = bigp.tile([C4, C4], F32, tag=f"ETn{bi}")
                nc.vector.tensor_scalar_mul(ETn[:], ET[:], rec[:, 0:1])
                etp = ps.tile([128, 1024], F32, tag="dwB")
                nc.tensor.transpose(etp[0:C4, 0:C4], ETn[:],
                                    id_f32[0:C4, 0:C4])
                nc.scalar.copy(Ebd[o:o + C4, o:o + C4], etp[0:C4, 0:C4])

            # ---------- out2 = attn @ v, both batches per matmul ----------
            for w in range(NW):
                y0 = w * WR
                o2st = stgp.tile([C2, WR * W], BF16, tag="o2st")
                for pr in range(2):
                    pc0 = pr * 1024
                    o2ps = ps.tile([128, 1024], F32, tag="cv", bufs=2)
                    for hf in range(2):
                        nc.tensor.matmul(
                            o2ps[0:C2, hf * 512:(hf + 1) * 512], Ebd[:],
                            vl[0:C2, y0 * W + pc0 + hf * 512:
                               y0 * W + pc0 + (hf + 1) * 512],
                            start=True, stop=True)
                    if pr == 0:
                        nc.scalar.copy(o2st[:, pc0:pc0 + 1024],
                                       o2ps[0:C2, :])
                    else:
                        nc.vector.tensor_copy(o2st[:, pc0:pc0 + 1024],
                                              o2ps[0:C2, :])
                o2st3 = o2st.rearrange("p (r w) -> p r w", w=W)
                hr = WR // 2
                for b in range(2):
                    nc.gpsimd.dma_start(
                        out[b:b + 1, 96:128, y0:y0 + hr, :],
                        o2st3[C4 * b:C4 * b + C4, 0:hr, :])
                    nc.gpsimd.dma_start(
                        out[b:b + 1, 96:128, y0 + hr:y0 + WR, :],
                        o2st3[C4 * b:C4 * b + C4, hr:WR, :])

        for _ in range(loops):
            one_pass()

    nc.compile()
    return nc


_NC_CACHE = None


def _get_nc():
    global _NC_CACHE
    if _NC_CACHE is None:
        _NC_CACHE = build_nc()
    return _NC_CACHE


def kernel(x, dw_w, dw_b, qkvl_w, qkvl_b):
    x = np.ascontiguousarray(np.asarray(x, dtype=np.float32))
    shared = {
        "dw_w": np.ascontiguousarray(np.asarray(dw_w, dtype=np.float32)),
        "dw_b": np.ascontiguousarray(np.asarray(dw_b, dtype=np.float32)),
        "qkvl_w": np.ascontiguousarray(np.asarray(qkvl_w, dtype=np.float32)),
        "qkvl_b": np.ascontiguousarray(np.asarray(qkvl_b, dtype=np.float32)),
    }
    nc = _get_nc()
    in_maps = [
        {"x": x[c * BPC:(c + 1) * BPC], **shared} for c in range(N_CORES)
    ]
    res = bass_utils.run_bass_kernel_spmd(nc, in_maps,
                                          core_ids=list(range(N_CORES)))
    return np.concatenate(
        [np.asarray(res.results[c]["out"]).astype(np.float32)
         for c in range(N_CORES)], axis=0)
